# revision 16
# baseline (speedup 1.0000x reference)
"""CrossAttentionPool forward on 8 TRN2 NeuronCores.

Reference computation (per batch b):
    q = lines[b] @ w_q.T ; k = videos[b] @ w_k.T
    scores = (q @ k.T) * D**-0.5, masked where video_mask==0
    out = softmax(scores, axis=-1) @ videos[b]

Strategy (data-parallel over batch, 4 batches/core):
    scores = lines @ W @ videos^T with W = (w_q.T @ w_k) * scale folded on host.
    All matmul operands are float32r (TF32-class, ~2e-4 rel err, bf16-speed at
    N>=256); hardware rounds internally so raw fp32 bits ship straight into
    f32r DRAM tensors. Host marshalling ships lines/videos already transposed
    (feature-major), so the device does zero transposes - the TensorEngine
    runs only the three productive matmul groups:
        u[d,v]      = sum_d' W[d,d'] videos[v,d']      (36 MMs, N=512)
        scores^T    = sum_d  u[d,v]  lines[l,d]        (24 MMs, N=512)
        e^T         = exp(scores^T + mask_bias[v])      (ScalarE, LUT)
        out[l,:]    = sum_v  e^T[v,l] [videos | 1 1]   (32 MMs, N=512/258)
    The two appended ones-columns give the softmax denominator in the same
    matmul; rows are scaled by its reciprocal during the PSUM->SBUF copy.
    No max-subtraction in softmax: scores are O(1) for randn-scale inputs and
    the mask enters as an exp bias of -50 (matching the reference's -1e9
    masking to ~1e-16 relative).
"""
import numpy as np
import concourse.bacc as bacc
import concourse.tile as tile
from concourse import mybir
from concourse.bass_utils import run_bass_kernel_spmd

N_CORES = 8
B, L, V, D = 32, 512, 128, 768
BPC = B // N_CORES          # batches per core
KC = D // 128               # 6 contraction chunks
LC = L // 128               # 4 line chunks
F32 = mybir.dt.float32
F32R = mybir.dt.float32r
BF16 = mybir.dt.bfloat16


def _body(tc, out_d, linesT_d, vT01_d, vT23_d, vones_d, maskb_d, wl_d):
    nc = tc.nc
    from contextlib import ExitStack
    with ExitStack() as ctx:
        const = ctx.enter_context(tc.tile_pool(name="const", bufs=1))
        persist = ctx.enter_context(tc.tile_pool(name="persist", bufs=1))
        etpool = ctx.enter_context(tc.tile_pool(name="etp", bufs=2))
        outpool = ctx.enter_context(tc.tile_pool(name="osb", bufs=6))
        rpool = ctx.enter_context(tc.tile_pool(name="rp", bufs=4))

        pp_st = ctx.enter_context(tc.tile_pool(name="pp_st", bufs=2, space="PSUM"))
        pp_u = ctx.enter_context(tc.tile_pool(name="pp_u", bufs=2, space="PSUM"))
        pp_o1 = ctx.enter_context(tc.tile_pool(name="pp_o1", bufs=2, space="PSUM"))
        pp_o2 = ctx.enter_context(tc.tile_pool(name="pp_o2", bufs=2, space="PSUM"))

        maskb = const.tile([128, BPC], F32)

        # critical path first: videos^T pair 0 (u-MM rhs) on Sync, weights on
        # Scalar's HWDGE queue in parallel. All inputs are host-prearranged to
        # the exact SBUF layout -> plain contiguous [128, X] DMAs.
        # vT pair tiles: [128, (c, 2, v)] (partition = d' within chunk c)
        vT01 = persist.tile([128, KC, 2 * V], BF16, tag="vT01")
        nc.sync.dma_start(vT01[:], vT01_d[:].rearrange("p (c w) -> p c w", w=2 * V))
        # wl m-major: wl_r[:, m, c, s] = WL[c*128+p, m*128+s]; u(m) needs
        # only slice m, so the first u matmuls start after 1/6 of the weights.
        wl_r = persist.tile([128, KC, KC, 128], BF16, tag="wlr")
        wl_v = wl_d[:].rearrange("p (m c s) -> p m c s", m=KC, c=KC)
        for m in range(KC):
            nc.scalar.dma_start(wl_r[:, m], wl_v[:, m])
        vT23 = persist.tile([128, KC, 2 * V], BF16, tag="vT23")
        nc.sync.dma_start(vT23[:], vT23_d[:].rearrange("p (c w) -> p c w", w=2 * V))

        # lines^T per batch: lT[b][:, c, l] (partition = d within chunk c)
        lT = [persist.tile([128, KC, L], BF16, tag=f"lT{b}", name=f"lT{b}")
              for b in range(BPC)]
        vbr = persist.tile([128, BPC, D + 2], BF16, tag="vbr")
        nc.sync.dma_start(lT[0][:],
                          linesT_d[0].rearrange("p (c w) -> p c w", w=L))
        nc.sync.dma_start(maskb[:], maskb_d[:])
        # videos natural + two ones columns: [v, (b, d+2)]
        nc.sync.dma_start(vbr[:], vones_d[:].rearrange("p (b w) -> p b w", w=D + 2))
        for b in (1, 2, 3):
            nc.sync.dma_start(lT[b][:],
                              linesT_d[b].rearrange("p (c w) -> p c w", w=L))

        # u pair tiles: [128, (m, 2, v)] (partition = d within chunk m)
        u01 = persist.tile([128, KC, 2 * V], BF16, tag="u01")
        u23 = persist.tile([128, KC, 2 * V], BF16, tag="u23")
        us = {0: u01, 1: u23}
        vs = {0: vT01, 1: vT23}

        # ---------------- u = W @ videos^T (per batch-pair, N=256) ----------
        for pair in (0, 1):
            for m in range(KC):
                pu = pp_u.tile([128, 256], F32)
                for c in range(KC):
                    nc.tensor.matmul(pu[:],
                                     wl_r[:, m, c],
                                     vs[pair][:, c],
                                     start=(c == 0), stop=(c == KC - 1))
                nc.vector.tensor_copy(us[pair][:, m], pu[:])

        # ---------------- per-batch: scores^T -> exp -> out ----------------
        for b in range(BPC):
            psT = pp_st.tile([128, 512], F32)
            ub = us[b // 2]
            for m in range(KC):
                nc.tensor.matmul(psT[:],
                                 ub[:, m, (b % 2) * V:(b % 2 + 1) * V],
                                 lT[b][:, m, :],
                                 start=(m == 0), stop=(m == KC - 1))
            eT = etpool.tile([128, 512], BF16)
            nc.scalar.activation(eT[:], psT[:], mybir.ActivationFunctionType.Exp,
                                 bias=maskb[:, b:b + 1])

            for i in range(LC):
                po1 = pp_o1.tile([128, 512], F32)
                nc.tensor.matmul(po1[:], eT[:, i * 128:(i + 1) * 128],
                                 vbr[:, b, 0:512], start=True, stop=True)
                po2 = pp_o2.tile([128, 258], F32)
                nc.tensor.matmul(po2[:], eT[:, i * 128:(i + 1) * 128],
                                 vbr[:, b, 512:D + 2], start=True, stop=True)
                rec = rpool.tile([128, 1], F32)
                nc.vector.reciprocal(rec[:], po2[:, 256:257])
                osb = outpool.tile([128, D], F32)
                if i % 2 == 0:
                    nc.scalar.mul(osb[:, 0:512], po1[:], rec[:])
                    nc.vector.tensor_scalar_mul(osb[:, 512:D], po2[:, 0:256],
                                                rec[:])
                else:
                    nc.vector.tensor_scalar_mul(osb[:, 0:512], po1[:], rec[:])
                    nc.scalar.mul(osb[:, 512:D], po2[:, 0:256], rec[:])
                nc.sync.dma_start(out_d[b, i * 128:(i + 1) * 128, :], osb[:])


_CACHE = {}


def _build():
    if "nc" in _CACHE:
        return _CACHE["nc"]
    nc = bacc.Bacc("TRN2", target_bir_lowering=False, debug=False,
                   num_devices=N_CORES)
    linesT_d = nc.dram_tensor("linesT", [BPC, 128, KC * L], BF16,
                              kind="ExternalInput").ap()
    vT01_d = nc.dram_tensor("vT01", [128, KC * 2 * V], BF16,
                            kind="ExternalInput").ap()
    vT23_d = nc.dram_tensor("vT23", [128, KC * 2 * V], BF16,
                            kind="ExternalInput").ap()
    vones_d = nc.dram_tensor("vones", [128, BPC * (D + 2)], BF16,
                             kind="ExternalInput").ap()
    maskb_d = nc.dram_tensor("maskb", [V, BPC], F32, kind="ExternalInput").ap()
    wl_d = nc.dram_tensor("wl", [128, KC * D], BF16, kind="ExternalInput").ap()
    out_d = nc.dram_tensor("out", [BPC, L, D], F32, kind="ExternalOutput").ap()
    with tile.TileContext(nc) as tc:
        _body(tc, out_d, linesT_d, vT01_d, vT23_d, vones_d, maskb_d, wl_d)
    nc.compile()
    _CACHE["nc"] = nc
    return nc


def _in_maps(lines, videos, video_mask, w_q, w_k):
    w_q = np.asarray(w_q, dtype=np.float32)
    w_k = np.asarray(w_k, dtype=np.float32)
    video_mask = np.asarray(video_mask)
    scale = np.float64(D) ** -0.5
    # scores = lines @ (w_q.T @ w_k * scale) @ videos^T; device wants WL[d', d] = W[d, d']
    WL = (scale * (w_k.astype(np.float64).T @ w_q.astype(np.float64))
          ).astype(np.float32)
    mask_bias = np.where(np.asarray(video_mask) == 0,
                         np.float32(-50.0), np.float32(0.0)).astype(np.float32)
    import ml_dtypes
    bf16 = ml_dtypes.bfloat16
    videos = np.asarray(videos, dtype=np.float32)
    lines = np.asarray(lines, dtype=np.float32)
    # vbr layout [v, (b, d+2)] per core
    vones = np.concatenate(
        [videos, np.ones((B, V, 2), dtype=np.float32)], axis=2).astype(bf16)
    vones = vones.reshape(N_CORES, BPC, V, D + 2).transpose(0, 2, 1, 3)
    vones = np.ascontiguousarray(vones.reshape(N_CORES, V, BPC * (D + 2)))
    # lT layout [b][p=d%128, (c=d//128, l)] per core
    linesT = lines.transpose(0, 2, 1).astype(bf16)          # [B, D, L]
    linesT = linesT.reshape(B, KC, 128, L).transpose(0, 2, 1, 3)
    linesT = np.ascontiguousarray(linesT.reshape(N_CORES, BPC, 128, KC * L))
    # vT pair layout [p=d'%128, (c, bpair, v)] per core
    videosT = videos.transpose(0, 2, 1).astype(bf16)        # [B, D, V]
    videosT = videosT.reshape(N_CORES, BPC, KC, 128, V).transpose(0, 3, 2, 1, 4)
    # -> [cores, 128, KC, BPC, V]; split pairs
    vT01 = np.ascontiguousarray(
        videosT[:, :, :, 0:2, :].reshape(N_CORES, 128, KC * 2 * V))
    vT23 = np.ascontiguousarray(
        videosT[:, :, :, 2:4, :].reshape(N_CORES, 128, KC * 2 * V))
    # wl layout [p=d'%128, (c=d'//128, d)]
    # [p, (m, c, s)] with wl[p, m, c, s] = WL[c*128+p, m*128+s]
    WLh = np.ascontiguousarray(
        WL.astype(bf16).reshape(KC, 128, KC, 128)
        .transpose(1, 2, 0, 3).reshape(128, KC * D))
    maps = []
    for c in range(N_CORES):
        sl = slice(c * BPC, (c + 1) * BPC)
        maps.append({
            "linesT": linesT[c],
            "vT01": vT01[c],
            "vT23": vT23[c],
            "vones": vones[c],
            "maskb": np.ascontiguousarray(mask_bias[sl].T),
            "wl": WLh,
        })
    return maps


def kernel(lines, videos, video_mask, w_q, w_k):
    nc = _build()
    maps = _in_maps(lines, videos, video_mask, w_q, w_k)
    res = run_bass_kernel_spmd(nc, maps, list(range(N_CORES)))
    out = np.concatenate([res.results[c]["out"] for c in range(N_CORES)], axis=0)
    return np.ascontiguousarray(out.astype(np.float32))


# revision 17
# speedup vs baseline: 1.0585x; 1.0585x over previous
"""CrossAttentionPool forward on 8 TRN2 NeuronCores.

Reference computation (per batch b):
    q = lines[b] @ w_q.T ; k = videos[b] @ w_k.T
    scores = (q @ k.T) * D**-0.5, masked where video_mask==0
    out = softmax(scores, axis=-1) @ videos[b]

Strategy (data-parallel over batch, 4 batches/core):
    scores = lines @ W @ videos^T with W = (w_q.T @ w_k) * scale folded on host.
    All matmul operands are float32r (TF32-class, ~2e-4 rel err, bf16-speed at
    N>=256); hardware rounds internally so raw fp32 bits ship straight into
    f32r DRAM tensors. Host marshalling ships lines/videos already transposed
    (feature-major), so the device does zero transposes - the TensorEngine
    runs only the three productive matmul groups:
        u[d,v]      = sum_d' W[d,d'] videos[v,d']      (36 MMs, N=512)
        scores^T    = sum_d  u[d,v]  lines[l,d]        (24 MMs, N=512)
        e^T         = exp(scores^T + mask_bias[v])      (ScalarE, LUT)
        out[l,:]    = sum_v  e^T[v,l] [videos | 1 1]   (32 MMs, N=512/258)
    The two appended ones-columns give the softmax denominator in the same
    matmul; rows are scaled by its reciprocal during the PSUM->SBUF copy.
    No max-subtraction in softmax: scores are O(1) for randn-scale inputs and
    the mask enters as an exp bias of -50 (matching the reference's -1e9
    masking to ~1e-16 relative).
"""
import numpy as np
import concourse.bacc as bacc
import concourse.tile as tile
from concourse import mybir
from concourse.bass_utils import run_bass_kernel_spmd

N_CORES = 8
B, L, V, D = 32, 512, 128, 768
BPC = B // N_CORES          # batches per core
KC = D // 128               # 6 contraction chunks
LC = L // 128               # 4 line chunks
F32 = mybir.dt.float32
F32R = mybir.dt.float32r
BF16 = mybir.dt.bfloat16


def _body(tc, out_d, linesT_d, vT01_d, vT23_d, vones_d, maskb_d, wl_d):
    nc = tc.nc
    from contextlib import ExitStack
    with ExitStack() as ctx:
        const = ctx.enter_context(tc.tile_pool(name="const", bufs=1))
        persist = ctx.enter_context(tc.tile_pool(name="persist", bufs=1))
        etpool = ctx.enter_context(tc.tile_pool(name="etp", bufs=2))
        outpool = ctx.enter_context(tc.tile_pool(name="osb", bufs=6))
        rpool = ctx.enter_context(tc.tile_pool(name="rp", bufs=4))

        pp_st = ctx.enter_context(tc.tile_pool(name="pp_st", bufs=2, space="PSUM"))
        pp_u = ctx.enter_context(tc.tile_pool(name="pp_u", bufs=2, space="PSUM"))
        pp_o1 = ctx.enter_context(tc.tile_pool(name="pp_o1", bufs=2, space="PSUM"))
        pp_o2 = ctx.enter_context(tc.tile_pool(name="pp_o2", bufs=2, space="PSUM"))

        maskb = const.tile([128, BPC], F32)

        # critical path first: videos^T pair 0 (u-MM rhs) on Sync, weights on
        # Scalar's HWDGE queue in parallel. All inputs are host-prearranged to
        # the exact SBUF layout -> plain contiguous [128, X] DMAs.
        # vT pair tiles: [128, (c, 2, v)] (partition = d' within chunk c)
        vT01 = persist.tile([128, KC, 2 * V], BF16, tag="vT01")
        nc.sync.dma_start(vT01[:], vT01_d[:].rearrange("p (c w) -> p c w", w=2 * V))
        # wl m-major: wl_r[:, m, c, s] = WL[c*128+p, m*128+s]; u(m) needs
        # only slice m, so the first u matmuls start after 1/6 of the weights.
        wl_r = persist.tile([128, KC, KC, 128], BF16, tag="wlr")
        wl_v = wl_d[:].rearrange("p (m c s) -> p m c s", m=KC, c=KC)
        for m in range(KC):
            nc.scalar.dma_start(wl_r[:, m], wl_v[:, m])
        vT23 = persist.tile([128, KC, 2 * V], BF16, tag="vT23")
        nc.sync.dma_start(vT23[:], vT23_d[:].rearrange("p (c w) -> p c w", w=2 * V))

        # lines^T per batch: lT[b][:, c, l] (partition = d within chunk c)
        lT = [persist.tile([128, KC, L], BF16, tag=f"lT{b}", name=f"lT{b}")
              for b in range(BPC)]
        vbr = persist.tile([128, BPC, D + 2], BF16, tag="vbr")
        nc.sync.dma_start(lT[0][:],
                          linesT_d[0].rearrange("p (c w) -> p c w", w=L))
        nc.sync.dma_start(maskb[:], maskb_d[:])
        # videos natural + two ones columns: [v, (b, d+2)]
        nc.sync.dma_start(vbr[:], vones_d[:].rearrange("p (b w) -> p b w", w=D + 2))
        for b in (1, 2, 3):
            nc.sync.dma_start(lT[b][:],
                              linesT_d[b].rearrange("p (c w) -> p c w", w=L))

        # u pair tiles: [128, (m, 2, v)] (partition = d within chunk m)
        u01 = persist.tile([128, KC, 2 * V], BF16, tag="u01")
        u23 = persist.tile([128, KC, 2 * V], BF16, tag="u23")
        us = {0: u01, 1: u23}
        vs = {0: vT01, 1: vT23}

        # ---------------- u = W @ videos^T (per batch-pair, N=256) ----------
        for pair in (0, 1):
            for m in range(KC):
                pu = pp_u.tile([128, 256], F32)
                for c in range(KC):
                    nc.tensor.matmul(pu[:],
                                     wl_r[:, m, c],
                                     vs[pair][:, c],
                                     start=(c == 0), stop=(c == KC - 1))
                nc.vector.tensor_copy(us[pair][:, m], pu[:])

        # ---------------- per-batch: scores^T -> exp -> out ----------------
        for b in range(BPC):
            psT = pp_st.tile([128, 512], F32)
            ub = us[b // 2]
            for m in range(KC):
                nc.tensor.matmul(psT[:],
                                 ub[:, m, (b % 2) * V:(b % 2 + 1) * V],
                                 lT[b][:, m, :],
                                 start=(m == 0), stop=(m == KC - 1))
            eT = etpool.tile([128, 512], BF16)
            # exp in l-chunk slices so the first out-matmuls start after 1/4
            for i in range(LC):
                nc.scalar.activation(eT[:, i * 128:(i + 1) * 128],
                                     psT[:, i * 128:(i + 1) * 128],
                                     mybir.ActivationFunctionType.Exp,
                                     bias=maskb[:, b:b + 1])

            for i in range(LC):
                po1 = pp_o1.tile([128, 512], F32)
                nc.tensor.matmul(po1[:], eT[:, i * 128:(i + 1) * 128],
                                 vbr[:, b, 0:512], start=True, stop=True)
                po2 = pp_o2.tile([128, 258], F32)
                nc.tensor.matmul(po2[:], eT[:, i * 128:(i + 1) * 128],
                                 vbr[:, b, 512:D + 2], start=True, stop=True)
                rec = rpool.tile([128, 1], F32)
                nc.vector.reciprocal(rec[:], po2[:, 256:257])
                osb = outpool.tile([128, D], F32)
                if i % 2 == 0:
                    nc.scalar.mul(osb[:, 0:512], po1[:], rec[:])
                    nc.vector.tensor_scalar_mul(osb[:, 512:D], po2[:, 0:256],
                                                rec[:])
                else:
                    nc.vector.tensor_scalar_mul(osb[:, 0:512], po1[:], rec[:])
                    nc.scalar.mul(osb[:, 512:D], po2[:, 0:256], rec[:])
                oeng = nc.sync if (b * LC + i) % 2 == 0 else nc.scalar
                oeng.dma_start(out_d[b, i * 128:(i + 1) * 128, :], osb[:])


_CACHE = {}


def _build():
    if "nc" in _CACHE:
        return _CACHE["nc"]
    nc = bacc.Bacc("TRN2", target_bir_lowering=False, debug=False,
                   num_devices=N_CORES)
    linesT_d = nc.dram_tensor("linesT", [BPC, 128, KC * L], BF16,
                              kind="ExternalInput").ap()
    vT01_d = nc.dram_tensor("vT01", [128, KC * 2 * V], BF16,
                            kind="ExternalInput").ap()
    vT23_d = nc.dram_tensor("vT23", [128, KC * 2 * V], BF16,
                            kind="ExternalInput").ap()
    vones_d = nc.dram_tensor("vones", [128, BPC * (D + 2)], BF16,
                             kind="ExternalInput").ap()
    maskb_d = nc.dram_tensor("maskb", [V, BPC], F32, kind="ExternalInput").ap()
    wl_d = nc.dram_tensor("wl", [128, KC * D], BF16, kind="ExternalInput").ap()
    out_d = nc.dram_tensor("out", [BPC, L, D], F32, kind="ExternalOutput").ap()
    with tile.TileContext(nc) as tc:
        _body(tc, out_d, linesT_d, vT01_d, vT23_d, vones_d, maskb_d, wl_d)
    nc.compile()
    _CACHE["nc"] = nc
    return nc


def _in_maps(lines, videos, video_mask, w_q, w_k):
    w_q = np.asarray(w_q, dtype=np.float32)
    w_k = np.asarray(w_k, dtype=np.float32)
    video_mask = np.asarray(video_mask)
    scale = np.float64(D) ** -0.5
    # scores = lines @ (w_q.T @ w_k * scale) @ videos^T; device wants WL[d', d] = W[d, d']
    WL = (scale * (w_k.astype(np.float64).T @ w_q.astype(np.float64))
          ).astype(np.float32)
    mask_bias = np.where(np.asarray(video_mask) == 0,
                         np.float32(-50.0), np.float32(0.0)).astype(np.float32)
    import ml_dtypes
    bf16 = ml_dtypes.bfloat16
    videos = np.asarray(videos, dtype=np.float32)
    lines = np.asarray(lines, dtype=np.float32)
    # vbr layout [v, (b, d+2)] per core
    vones = np.concatenate(
        [videos, np.ones((B, V, 2), dtype=np.float32)], axis=2).astype(bf16)
    vones = vones.reshape(N_CORES, BPC, V, D + 2).transpose(0, 2, 1, 3)
    vones = np.ascontiguousarray(vones.reshape(N_CORES, V, BPC * (D + 2)))
    # lT layout [b][p=d%128, (c=d//128, l)] per core
    linesT = lines.transpose(0, 2, 1).astype(bf16)          # [B, D, L]
    linesT = linesT.reshape(B, KC, 128, L).transpose(0, 2, 1, 3)
    linesT = np.ascontiguousarray(linesT.reshape(N_CORES, BPC, 128, KC * L))
    # vT pair layout [p=d'%128, (c, bpair, v)] per core
    videosT = videos.transpose(0, 2, 1).astype(bf16)        # [B, D, V]
    videosT = videosT.reshape(N_CORES, BPC, KC, 128, V).transpose(0, 3, 2, 1, 4)
    # -> [cores, 128, KC, BPC, V]; split pairs
    vT01 = np.ascontiguousarray(
        videosT[:, :, :, 0:2, :].reshape(N_CORES, 128, KC * 2 * V))
    vT23 = np.ascontiguousarray(
        videosT[:, :, :, 2:4, :].reshape(N_CORES, 128, KC * 2 * V))
    # wl layout [p=d'%128, (c=d'//128, d)]
    # [p, (m, c, s)] with wl[p, m, c, s] = WL[c*128+p, m*128+s]
    WLh = np.ascontiguousarray(
        WL.astype(bf16).reshape(KC, 128, KC, 128)
        .transpose(1, 2, 0, 3).reshape(128, KC * D))
    maps = []
    for c in range(N_CORES):
        sl = slice(c * BPC, (c + 1) * BPC)
        maps.append({
            "linesT": linesT[c],
            "vT01": vT01[c],
            "vT23": vT23[c],
            "vones": vones[c],
            "maskb": np.ascontiguousarray(mask_bias[sl].T),
            "wl": WLh,
        })
    return maps


def kernel(lines, videos, video_mask, w_q, w_k):
    nc = _build()
    maps = _in_maps(lines, videos, video_mask, w_q, w_k)
    res = run_bass_kernel_spmd(nc, maps, list(range(N_CORES)))
    out = np.concatenate([res.results[c]["out"] for c in range(N_CORES)], axis=0)
    return np.ascontiguousarray(out.astype(np.float32))
